# revision 30
# baseline (speedup 1.0000x reference)
"""Causal multi-head attention on 8 Trainium2 NeuronCores (Bass/Tile).

Problem: B=4, S=2048, D=1024, H=16 heads (HD=64), fp32, causal softmax.

Sharding (tensor parallel over heads): core c owns heads {2c, 2c+1}:
  - Wq/Wk/Wv column slices [D, 128], Wo row slice [128, D]
  - each core computes Q/K/V for its heads over the full batch, runs
    attention for its 8 (batch, head) pairs, and produces a partial
    output projection in bf16, laid out transposed as [B, D, S]; the
    host sums the 8 partials in fp32, transposes to [B, S, D], + bo.

Device dataflow (all matmuls bf16 in / fp32 PSUM accumulate):
  - x is pre-transposed on host to xt[D, B*S] bf16 so Q/K/V projections
    are weight-stationary: Q^T[c, s] = sum_d Wq[d, c] xt[d, s]. V is
    produced transposed and relaid to [s, c] via PE transpose + DVE
    copy.
  - scores are computed transposed, S^T[k, q], with the two heads packed
    onto disjoint PE row-groups (head0 K=64 at partitions 0-63, head1 at
    64-127; confirmed ~80% concurrent on HW); one ACT exp call covers
    both heads' [128, 2, 512] chunk.
  - exp'd scores stream as the AV matmul's moving operand with [V | 1]
    stationary; the ones-column gives softmax denominators in psum row
    64. softmax skips max-subtraction (scaled causal scores are ~N(0,1)).
  - causal masking: sub-diagonal blocks are never computed; diagonal
    128x128 blocks get a post-exp multiplicative 0/1 mask (Pool).
  - denominators: AV evictions drop psum row 64 into a [1, S] row per
    head; ACT-table reciprocal on the row, then a gpsimd
    partition_broadcast replicates it across the head's 64 partitions
    (replaces the old selector-matmul broadcast, which burned ~17us of
    PE), then one in-place DVE multiply per 512-token segment covering
    both heads.
  - out^T[d, s] = sum_c Wo[c, d] ctxt[c, s]: weight-stationary out-proj
    in [128, 512] units; psum evictions alternate DVE/ACT.
  - SOFTWARE PIPELINE (engines execute streams in program order): the
    attention of batch b is emitted interleaved with the out-projection
    of batch b-1 (units zipped between score blocks) and the QKV
    projection chunks of batch b+1 (one 512-token chunk per q-chunk
    slot), so the PE never idles while ACT works through exp and the
    clock ramp (0.65/1.2/2.4 GHz pstates, 3us to max) stays hot.
    Batch 0's QKV runs standalone with the xt DMA split into 512-token
    slices so the first matmul starts ~1.5us in; batch 3 normalizes and
    projects per q-chunk right behind its AV to shrink the tail.
"""

import numpy as np
import ml_dtypes

import concourse.bass as bass
import concourse.mybir as mybir
import concourse.tile as tile
from concourse import bacc
from concourse import bass_utils

B, S, D, H, HD = 4, 2048, 1024, 16, 64
N_CORES = 8
HPC = H // N_CORES          # heads per core = 2
CSL = HPC * HD              # per-core channel slice = 128
NSB = S // 128              # 16 s-blocks per sequence
NCH = D // 128              # 8 contraction chunks
NQC = S // 512              # 4 q-chunks of 512
BF16 = mybir.dt.bfloat16
F32 = mybir.dt.float32
EXP = mybir.ActivationFunctionType.Exp
SCALE = 1.0 / float(np.sqrt(HD))

_CACHE: dict = {}
LAST_RESULTS = None  # BassKernelResults of the most recent run (for test.py)


def _build():
    nc = bacc.Bacc("TRN2", target_bir_lowering=False, debug=False,
                   num_devices=N_CORES)
    xt_d = nc.dram_tensor("xt", [D, B * S], BF16, kind="ExternalInput")
    wq_d = nc.dram_tensor("wq", [D, CSL], BF16, kind="ExternalInput")
    wk_d = nc.dram_tensor("wk", [D, CSL], BF16, kind="ExternalInput")
    wv_d = nc.dram_tensor("wv", [D, CSL], BF16, kind="ExternalInput")
    wo_d = nc.dram_tensor("wo", [CSL, D], BF16, kind="ExternalInput")
    tri_d = nc.dram_tensor("tri", [128, 128], BF16, kind="ExternalInput")
    ident_d = nc.dram_tensor("ident", [128, 128], BF16, kind="ExternalInput")
    out_d = nc.dram_tensor("out", [B, D, S], BF16, kind="ExternalOutput")

    with tile.TileContext(nc) as tc:
        with (
            tc.tile_pool(name="const", bufs=1) as cpool,
            tc.tile_pool(name="xt", bufs=2) as xtpool,
            tc.tile_pool(name="seq", bufs=2) as seqpool,
            tc.tile_pool(name="p", bufs=1) as ppool,
            tc.tile_pool(name="small", bufs=4) as small,
            tc.tile_pool(name="outsb", bufs=6) as outsb,
            tc.tile_pool(name="ps_s", bufs=2, space="PSUM") as ps_s,
            tc.tile_pool(name="ps_mm", bufs=2, space="PSUM") as ps_mm,
            tc.tile_pool(name="ps_cacc", bufs=2, space="PSUM") as ps_cacc,
        ):
            wq_sb = cpool.tile([128, NCH, CSL], BF16)
            wk_sb = cpool.tile([128, NCH, CSL], BF16)
            wv_sb = cpool.tile([128, NCH, CSL], BF16)
            wo_sb = cpool.tile([128, NCH, 128], BF16)
            tri_sb = cpool.tile([128, 128], BF16)
            ident_sb = cpool.tile([128, 128], BF16)
            nc.sync.dma_start(wq_sb[:], wq_d.ap().rearrange("(c p) m -> p c m", p=128))
            nc.sync.dma_start(wk_sb[:], wk_d.ap().rearrange("(c p) m -> p c m", p=128))
            nc.sync.dma_start(wv_sb[:], wv_d.ap().rearrange("(c p) m -> p c m", p=128))
            nc.sync.dma_start(wo_sb[:], wo_d.ap().rearrange("p (j m) -> p j m", m=128))
            nc.sync.dma_start(tri_sb[:], tri_d.ap())
            nc.sync.dma_start(ident_sb[:], ident_d.ap())

            def load_xt(b, fine=False):
                xt_sb = xtpool.tile([128, NCH, S], BF16)
                if fine:
                    # sc-major 512-col slices: chunk c of slice 0 lands
                    # ~0.4us in, so batch 0's QKV starts almost at once.
                    for sc in range(NQC):
                        for c in range(NCH):
                            nc.sync.dma_start(
                                xt_sb[:, c, sc * 512:(sc + 1) * 512],
                                xt_d.ap()[c * 128:(c + 1) * 128,
                                          b * S + sc * 512:b * S + (sc + 1) * 512])
                else:
                    for c in range(NCH):
                        nc.sync.dma_start(
                            xt_sb[:, c, :],
                            xt_d.ap()[c * 128:(c + 1) * 128, b * S:(b + 1) * S])
                return xt_sb

            def new_seq(b):
                st = {
                    "qt": seqpool.tile([128, S], BF16, tag="qt", name="qt"),
                    "kt": seqpool.tile([128, S], BF16, tag="kt", name="kt"),
                    "vt": seqpool.tile([128, S], BF16, tag="vt", name="vt"),
                    "v": seqpool.tile([128, NSB, HPC, HD + 1], BF16, tag="v",
                                      name="v"),
                }
                nc.vector.memset(st["v"][:, :, :, HD:HD + 1], 1.0)
                return st

            def qkv_chunk(st, xt_sb, sc):
                """Q/K/V projection + V relayout for one 512-token chunk."""
                qkacc = ps_s.tile([128, 2, 512], F32, tag="s", name="qkacc")
                vacc = ps_mm.tile([128, 512], F32, tag="mm", name="vacc")
                for c in range(NCH):
                    for i, w_sb in enumerate((wq_sb, wk_sb)):
                        nc.tensor.matmul(
                            qkacc[:, i, :],
                            w_sb[:, c, :],
                            xt_sb[:, c, sc * 512:(sc + 1) * 512],
                            start=(c == 0), stop=(c == NCH - 1))
                    nc.tensor.matmul(
                        vacc[:],
                        wv_sb[:, c, :],
                        xt_sb[:, c, sc * 512:(sc + 1) * 512],
                        start=(c == 0), stop=(c == NCH - 1))
                for i, key in enumerate(("qt", "kt")):
                    nc.vector.tensor_copy(
                        st[key][:, sc * 512:(sc + 1) * 512], qkacc[:, i, :])
                nc.vector.tensor_copy(
                    st["vt"][:, sc * 512:(sc + 1) * 512], vacc[:])
                for i in range(4):
                    sb = sc * 4 + i
                    tp = ps_mm.tile([128, 128], BF16, tag="mm")
                    nc.tensor.transpose(
                        tp[:], st["vt"][:, sb * 128:(sb + 1) * 128], ident_sb[:])
                    nc.vector.tensor_copy(
                        st["v"][:, sb, :, 0:HD],
                        tp[:].rearrange("p (h e) -> p h e", h=HPC))

            # p tiles alternate between two tags (qc0/qc2 and qc1/qc3) so
            # only ~2 q-chunks of exp'd scores are resident at once.
            PTAGS = {0: ("pA", 12), 1: ("pB", 16), 2: ("pA", 12), 3: ("pB", 16)}

            def attn_scores(st, qc, fillers):
                """Score blocks for q-chunk qc; after each block, pop one
                deferred filler (out-proj unit closure) into the PE stream."""
                qt, kt = st["qt"], st["kt"]
                nki = 4 * qc + 4   # causal: k-blocks 0 .. 4qc+3
                tag, maxk = PTAGS[qc]
                p = ppool.tile([128, maxk, HPC, 512], BF16, tag=tag, name=tag)
                for ki in range(nki):
                    off = max(0, ki * 128 - qc * 512)
                    sacc = ps_s.tile([128, 2, 512], F32, tag="s")
                    for h in range(HPC):
                        nc.tensor.matmul(
                            sacc[:, h, off:512],
                            kt[h * HD:(h + 1) * HD, ki * 128:(ki + 1) * 128],
                            qt[h * HD:(h + 1) * HD, qc * 512 + off:(qc + 1) * 512],
                            start=True, stop=True)
                    nc.scalar.activation(
                        p[:, ki, :, off:512],
                        sacc[:, :, off:512], EXP, scale=SCALE)
                    if ki >= 4 * qc:  # diagonal: post-exp 0/1 mask per head
                        for h in range(HPC):
                            nc.gpsimd.tensor_mul(
                                p[:, ki, h, off:off + 128],
                                p[:, ki, h, off:off + 128],
                                tri_sb[:])
                    if fillers:
                        fillers.pop(0)()
                return p

            def attn_av(st, ctxt, drows, p, qc, fillers):
                nki = 4 * qc + 4
                caccs = [ps_cacc.tile([HD + 1, 512], F32, tag=f"c{h}",
                                      name=f"cacc{h}", bufs=1)
                         for h in range(HPC)]
                for ki in range(nki):
                    off = max(0, ki * 128 - qc * 512)
                    for h in range(HPC):
                        nc.tensor.matmul(
                            caccs[h][:, off:512],
                            st["v"][:, ki, h, :],
                            p[:, ki, h, off:512],
                            start=(ki == 0), stop=(ki == nki - 1))
                    if fillers:
                        fillers.pop(0)()
                for h in range(HPC):
                    nc.vector.tensor_copy(
                        drows[h][0:1, qc * 512:(qc + 1) * 512],
                        caccs[h][HD:HD + 1, :])
                    nc.vector.tensor_copy(
                        ctxt[h * HD:(h + 1) * HD, qc * 512:(qc + 1) * 512],
                        caccs[h][0:HD, :])

            def recip_rows(drows_h, rbf_h, c0, c1):
                # ACT-table reciprocal (~1e-5 rel err, fine for softmax
                # denominators that land in bf16 anyway). bass's
                # activation() hard-blocks Reciprocal, so emit directly.
                eng = nc.scalar
                ins = [eng.lower_ap(drows_h[0:1, c0:c1]),
                       mybir.ImmediateValue(dtype=mybir.dt.float32, value=0.0),
                       mybir.ImmediateValue(dtype=mybir.dt.float32, value=1.0),
                       mybir.ImmediateValue(dtype=mybir.dt.float32, value=0.0)]
                eng.add_instruction(mybir.InstActivation(
                    name=nc.get_next_instruction_name(),
                    func=mybir.ActivationFunctionType.Reciprocal,
                    ins=ins, outs=[eng.lower_ap(rbf_h[0:1, c0:c1])]))

            def bcast_recip(rbc, rb1, rbf):
                # replicate each head's recip row across its 64 partitions
                # (gpsimd ucode, ~2us each; only used for b<3 where the
                # consumer is a whole batch away). partition_broadcast can
                # only write at base partition 0, so head 1 goes through a
                # base-0 scratch and a partition-moving DMA.
                nc.gpsimd.partition_broadcast(rbc[0:HD, :], rbf[0][0:1, :])
                nc.gpsimd.partition_broadcast(rb1[:], rbf[1][0:1, :])
                nc.sync.dma_start(rbc[HD:128, :], rb1[:])

            def norm_seg(ctxt, rbc, qc):
                # both heads at once; SBUF-only so it runs on Pool
                sl = ctxt[:, qc * 512:(qc + 1) * 512]
                nc.gpsimd.tensor_mul(sl, sl, rbc[:, qc * 512:(qc + 1) * 512])

            def norm_seg_psum(ctxt, rbf, qc):
                # batch-3 tail: low-latency rank-1 PE broadcast of the recip
                # row (stationary = ones column from tri row 0, K=1), then
                # per-head DVE multiplies straight off psum (DVE tolerates
                # the psum->sbuf partition shift).
                for h in range(HPC):
                    bc = ps_mm.tile([128, 512], F32, tag="mm", name="bc")
                    nc.tensor.matmul(
                        bc[0:HD, :], tri_sb[0:1, 0:HD],
                        rbf[h][0:1, qc * 512:(qc + 1) * 512],
                        start=True, stop=True)
                    sl = ctxt[h * HD:(h + 1) * HD, qc * 512:(qc + 1) * 512]
                    nc.vector.tensor_mul(sl, sl, bc[0:HD, :])

            def outproj_unit(ctxt, b, j, seg):
                oacc = ps_mm.tile([128, 512], F32, tag="mm")
                nc.tensor.matmul(oacc[:],
                                 wo_sb[:, j, :],
                                 ctxt[:, seg * 512:(seg + 1) * 512],
                                 start=True, stop=True)
                osb = outsb.tile([128, 512], BF16, tag="o")
                nc.vector.tensor_copy(osb[:], oacc[:])
                nc.sync.dma_start(
                    out_d.ap()[b, j * 128:(j + 1) * 128,
                               seg * 512:(seg + 1) * 512],
                    osb[:])

            def outproj_fillers(ctxt, b):
                """32 deferred out-proj unit closures for batch b; psum
                evictions alternate DVE / ACT (Pool has no PSUM port)."""
                return [lambda ctxt=ctxt, b=b, j=j, seg=seg:
                        outproj_unit(ctxt, b, j, seg)
                        for seg in range(NQC) for j in range(NCH)]

            # ---- prologue: batch 0 QKV standalone ----
            xt_sb = load_xt(0, fine=True)
            xt_next = load_xt(1)
            seqs = {0: new_seq(0)}
            for sc in range(NQC):
                qkv_chunk(seqs[0], xt_sb, sc)
            xt_sb = xt_next

            prev = None   # (ctxt of b-1, b-1) with outproj not yet emitted
            for b in range(B):
                st = seqs.pop(b)
                ctxt = seqpool.tile([128, S], BF16, tag="ctxt")
                drows = [small.tile([1, S], F32, tag="drows0", name="drows0",
                                    bufs=1),
                         small.tile([1, S], F32, tag="drows1", name="drows1",
                                    bufs=1)]
                rbf = [small.tile([1, S], BF16, tag="rbf0", name="rbf0",
                                  bufs=1),
                       small.tile([1, S], BF16, tag="rbf1", name="rbf1",
                                  bufs=1)]
                rbc = small.tile([128, S], BF16, tag="rbc", name="rbc",
                                 bufs=1)
                rb1 = small.tile([HD, S], BF16, tag="rb1", name="rb1",
                                 bufs=1)
                fillers = outproj_fillers(prev[0], prev[1]) if prev else []

                def tail_pre(qc):
                    # batch 3: normalize q-chunk qc right after its AV and
                    # queue its out-proj units as fillers so the epilogue
                    # isn't one serial chain at the very end.
                    for h in range(HPC):
                        recip_rows(drows[h], rbf[h], qc * 512, (qc + 1) * 512)
                    norm_seg_psum(ctxt, rbf, qc)
                    fillers.extend(
                        lambda j=j, qc=qc: outproj_unit(ctxt, b, j, qc)
                        for j in range(NCH))

                pending = None   # (p, qc) with scores emitted, AV not yet
                for qc in range(NQC):
                    if b + 1 < B:
                        if qc == 0:
                            seqs[b + 1] = new_seq(b + 1)
                        qkv_chunk(seqs[b + 1], xt_sb, qc)
                    p = attn_scores(st, qc, fillers)
                    if pending is not None:
                        attn_av(st, ctxt, drows, pending[0], pending[1], fillers)
                        if b == B - 1:
                            tail_pre(pending[1])
                    pending = (p, qc)
                    if qc == 2 and b + 2 < B:
                        xt_next = load_xt(b + 2)
                attn_av(st, ctxt, drows, pending[0], pending[1], fillers)
                if b == B - 1:
                    tail_pre(pending[1])
                    for f in fillers:
                        f()
                else:
                    for f in fillers:   # leftover out-proj units of b-1
                        f()
                    for h in range(HPC):
                        recip_rows(drows[h], rbf[h], 0, S)
                    bcast_recip(rbc, rb1, rbf)
                    for qc in range(NQC):
                        norm_seg(ctxt, rbc, qc)
                    prev = (ctxt, b)
                    xt_sb = xt_next
    nc.compile()
    return nc


def _prep_inputs(x, Wq, Wk, Wv, Wo):
    bf16 = ml_dtypes.bfloat16
    xt = np.ascontiguousarray(
        np.asarray(x, dtype=np.float32).reshape(B * S, D).T).astype(bf16)
    k = np.arange(128)[:, None]
    q = np.arange(128)[None, :]
    tri = (q >= k).astype(np.float32).astype(bf16)   # allowed = q >= k
    Wq = np.asarray(Wq, dtype=np.float32)
    Wk = np.asarray(Wk, dtype=np.float32)
    Wv = np.asarray(Wv, dtype=np.float32)
    Wo = np.asarray(Wo, dtype=np.float32)
    in_maps = []
    for c in range(N_CORES):
        sl = slice(c * CSL, (c + 1) * CSL)
        in_maps.append({
            "xt": xt,
            "wq": np.ascontiguousarray(Wq[:, sl]).astype(bf16),
            "wk": np.ascontiguousarray(Wk[:, sl]).astype(bf16),
            "wv": np.ascontiguousarray(Wv[:, sl]).astype(bf16),
            "wo": np.ascontiguousarray(Wo[sl, :]).astype(bf16),
            "tri": tri,
            "ident": np.eye(128, dtype=np.float32).astype(bf16),
        })
    return in_maps


def kernel(x, Wq, Wk, Wv, Wo, bo):
    global LAST_RESULTS
    if "nc" not in _CACHE:
        _CACHE["nc"] = _build()
    nc = _CACHE["nc"]
    in_maps = _prep_inputs(x, Wq, Wk, Wv, Wo)
    res = bass_utils.run_bass_kernel_spmd(
        nc, in_maps, core_ids=list(range(N_CORES)))
    LAST_RESULTS = res
    acc = np.zeros((B, D, S), dtype=np.float32)
    for r in res.results:
        acc += r["out"].astype(np.float32)
    out = np.ascontiguousarray(acc.transpose(0, 2, 1))
    out += np.asarray(bo, dtype=np.float32)
    return out


if __name__ == "__main__":
    rng = np.random.default_rng(0)
    scale = 1.0 / np.sqrt(D)
    ins = {
        "x": rng.standard_normal((B, S, D), dtype=np.float32),
        "Wq": rng.standard_normal((D, D), dtype=np.float32) * scale,
        "Wk": rng.standard_normal((D, D), dtype=np.float32) * scale,
        "Wv": rng.standard_normal((D, D), dtype=np.float32) * scale,
        "Wo": rng.standard_normal((D, D), dtype=np.float32) * scale,
        "bo": np.zeros(D, dtype=np.float32),
    }
    out = kernel(**ins)
    print("kernel output:", out.shape, out.dtype, float(np.abs(out).mean()))


# revision 36
# speedup vs baseline: 1.0332x; 1.0332x over previous
"""Causal multi-head attention on 8 Trainium2 NeuronCores (Bass/Tile).

Problem: B=4, S=2048, D=1024, H=16 heads (HD=64), fp32, causal softmax.

Sharding (tensor parallel over heads): core c owns heads {2c, 2c+1}:
  - Wq/Wk/Wv column slices [D, 128], Wo row slice [128, D]
  - each core computes Q/K/V for its heads over the full batch, runs
    attention for its 8 (batch, head) pairs, and produces a partial
    output projection in bf16, laid out transposed as [B, D, S]; the
    host sums the 8 partials in fp32, transposes to [B, S, D], + bo.

Device dataflow (all matmuls bf16 in / fp32 PSUM accumulate):
  - x is pre-transposed on host to xt[D, B*S] bf16 so Q/K/V projections
    are weight-stationary: Q^T[c, s] = sum_d Wq[d, c] xt[d, s]. V is
    produced transposed and relaid to [s, c] via PE transpose + DVE
    copy.
  - scores are computed transposed, S^T[k, q], with the two heads packed
    onto disjoint PE row-groups (head0 K=64 at partitions 0-63, head1 at
    64-127; confirmed ~80% concurrent on HW); one ACT exp call covers
    both heads' [128, 2, 512] chunk.
  - exp'd scores stream as the AV matmul's moving operand with [V | 1]
    stationary; the ones-column gives softmax denominators in psum row
    64. softmax skips max-subtraction (scaled causal scores are ~N(0,1)).
  - causal masking: sub-diagonal blocks are never computed; diagonal
    128x128 blocks get a post-exp multiplicative 0/1 mask (Pool).
  - denominators: AV evictions drop psum row 64 into a [1, S] row per
    head; one ACT-table reciprocal per (head, batch) on the row (the
    exp<->recip ACT table swap costs ~2.6us, so strictly once per
    batch), then a rank-1 PE matmul (ones-column stationary, K=1)
    broadcasts the recip row to 64 psum partitions and a DVE multiply
    normalizes ctxt in place. The normalize units ride the filler queue
    so the recip latency hides under the next batch's QKV.
  - out^T[d, s] = sum_c Wo[c, d] ctxt[c, s]: weight-stationary out-proj
    in [128, 512] units; psum evictions alternate DVE/ACT.
  - SOFTWARE PIPELINE (engines execute streams in program order): the
    attention of batch b is emitted interleaved with the out-projection
    of batch b-1 (units zipped between score blocks) and the QKV
    projection chunks of batch b+1 (one 512-token chunk per q-chunk
    slot), so the PE never idles while ACT works through exp and the
    clock ramp (0.65/1.2/2.4 GHz pstates, 3us to max) stays hot.
    Batch 0's QKV runs standalone with the xt DMA split into 512-token
    slices so the first matmul starts ~1.5us in; batch 3 normalizes and
    projects per q-chunk right behind its AV to shrink the tail.
"""

import numpy as np
import ml_dtypes

import concourse.bass as bass
import concourse.mybir as mybir
import concourse.tile as tile
from concourse import bacc
from concourse import bass_utils

B, S, D, H, HD = 4, 2048, 1024, 16, 64
N_CORES = 8
HPC = H // N_CORES          # heads per core = 2
CSL = HPC * HD              # per-core channel slice = 128
NSB = S // 128              # 16 s-blocks per sequence
NCH = D // 128              # 8 contraction chunks
NQC = S // 512              # 4 q-chunks of 512
BF16 = mybir.dt.bfloat16
F32 = mybir.dt.float32
EXP = mybir.ActivationFunctionType.Exp
SCALE = 1.0 / float(np.sqrt(HD))

_CACHE: dict = {}
LAST_RESULTS = None  # BassKernelResults of the most recent run (for test.py)


def _build():
    nc = bacc.Bacc("TRN2", target_bir_lowering=False, debug=False,
                   num_devices=N_CORES)
    xt_d = nc.dram_tensor("xt", [D, B * S], BF16, kind="ExternalInput")
    wq_d = nc.dram_tensor("wq", [D, CSL], BF16, kind="ExternalInput")
    wk_d = nc.dram_tensor("wk", [D, CSL], BF16, kind="ExternalInput")
    wv_d = nc.dram_tensor("wv", [D, CSL], BF16, kind="ExternalInput")
    wo_d = nc.dram_tensor("wo", [CSL, D], BF16, kind="ExternalInput")
    tri_d = nc.dram_tensor("tri", [128, 128], BF16, kind="ExternalInput")
    ident_d = nc.dram_tensor("ident", [128, 128], BF16, kind="ExternalInput")
    out_d = nc.dram_tensor("out", [B, D, S], BF16, kind="ExternalOutput")

    with tile.TileContext(nc) as tc:
        with (
            tc.tile_pool(name="const", bufs=1) as cpool,
            tc.tile_pool(name="xt", bufs=2) as xtpool,
            tc.tile_pool(name="seq", bufs=2) as seqpool,
            tc.tile_pool(name="p", bufs=1) as ppool,
            tc.tile_pool(name="small", bufs=4) as small,
            tc.tile_pool(name="outsb", bufs=8) as outsb,
            tc.tile_pool(name="ps_s", bufs=2, space="PSUM") as ps_s,
            tc.tile_pool(name="ps_mm", bufs=2, space="PSUM") as ps_mm,
            tc.tile_pool(name="ps_cacc", bufs=2, space="PSUM") as ps_cacc,
        ):
            wq_sb = cpool.tile([128, NCH, CSL], BF16)
            wk_sb = cpool.tile([128, NCH, CSL], BF16)
            wv_sb = cpool.tile([128, NCH, CSL], BF16)
            wo_sb = cpool.tile([128, NCH, 128], BF16)
            tri_sb = cpool.tile([128, 128], BF16)
            ident_sb = cpool.tile([128, 128], BF16)
            # per-chunk weight DMAs so the first QKV matmul only waits on
            # chunk 0 of each weight, not the whole [D, CSL] transfer
            for c in range(NCH):
                for w_sb, w_d in ((wq_sb, wq_d), (wk_sb, wk_d), (wv_sb, wv_d)):
                    nc.sync.dma_start(
                        w_sb[:, c, :], w_d.ap()[c * 128:(c + 1) * 128, :])
            nc.sync.dma_start(wo_sb[:], wo_d.ap().rearrange("p (j m) -> p j m", m=128))
            nc.sync.dma_start(tri_sb[:], tri_d.ap())
            nc.sync.dma_start(ident_sb[:], ident_d.ap())

            def load_xt(b, fine=False):
                xt_sb = xtpool.tile([128, NCH, S], BF16)
                if fine:
                    # sc-major 512-col slices: chunk c of slice 0 lands
                    # ~0.4us in, so batch 0's QKV starts almost at once.
                    for sc in range(NQC):
                        for c in range(NCH):
                            nc.sync.dma_start(
                                xt_sb[:, c, sc * 512:(sc + 1) * 512],
                                xt_d.ap()[c * 128:(c + 1) * 128,
                                          b * S + sc * 512:b * S + (sc + 1) * 512])
                else:
                    for c in range(NCH):
                        nc.sync.dma_start(
                            xt_sb[:, c, :],
                            xt_d.ap()[c * 128:(c + 1) * 128, b * S:(b + 1) * S])
                return xt_sb

            def new_seq(b):
                st = {
                    "qt": seqpool.tile([128, S], BF16, tag="qt", name="qt"),
                    "kt": seqpool.tile([128, S], BF16, tag="kt", name="kt"),
                    "vt": seqpool.tile([128, S], BF16, tag="vt", name="vt"),
                    "v": seqpool.tile([128, NSB, HPC, HD + 1], BF16, tag="v",
                                      name="v"),
                }
                nc.vector.memset(st["v"][:, :, :, HD:HD + 1], 1.0)
                return st

            def qkv_chunk(st, xt_sb, sc):
                """Q/K/V projection + V relayout for one 512-token chunk."""
                qkacc = ps_s.tile([128, 2, 512], F32, tag="s", name="qkacc")
                vacc = ps_mm.tile([128, 512], F32, tag="mm", name="vacc")
                for c in range(NCH):
                    for i, w_sb in enumerate((wq_sb, wk_sb)):
                        nc.tensor.matmul(
                            qkacc[:, i, :],
                            w_sb[:, c, :],
                            xt_sb[:, c, sc * 512:(sc + 1) * 512],
                            start=(c == 0), stop=(c == NCH - 1))
                    nc.tensor.matmul(
                        vacc[:],
                        wv_sb[:, c, :],
                        xt_sb[:, c, sc * 512:(sc + 1) * 512],
                        start=(c == 0), stop=(c == NCH - 1))
                for i, key in enumerate(("qt", "kt")):
                    nc.vector.tensor_copy(
                        st[key][:, sc * 512:(sc + 1) * 512], qkacc[:, i, :])
                nc.vector.tensor_copy(
                    st["vt"][:, sc * 512:(sc + 1) * 512], vacc[:])
                for i in range(4):
                    sb = sc * 4 + i
                    tp = ps_mm.tile([128, 128], BF16, tag="mm")
                    nc.tensor.transpose(
                        tp[:], st["vt"][:, sb * 128:(sb + 1) * 128], ident_sb[:])
                    nc.vector.tensor_copy(
                        st["v"][:, sb, :, 0:HD],
                        tp[:].rearrange("p (h e) -> p h e", h=HPC))

            # p tiles alternate between two tags (qc0/qc2 and qc1/qc3) so
            # only ~2 q-chunks of exp'd scores are resident at once.
            PTAGS = {0: ("pA", 12), 1: ("pB", 16), 2: ("pA", 12), 3: ("pB", 16)}

            def attn_scores(st, qc, fillers):
                """Score blocks for q-chunk qc; after each block, pop one
                deferred filler (out-proj unit closure) into the PE stream."""
                qt, kt = st["qt"], st["kt"]
                nki = 4 * qc + 4   # causal: k-blocks 0 .. 4qc+3
                tag, maxk = PTAGS[qc]
                p = ppool.tile([128, maxk, HPC, 512], BF16, tag=tag, name=tag)
                for ki in range(nki):
                    off = max(0, ki * 128 - qc * 512)
                    sacc = ps_s.tile([128, 2, 512], F32, tag="s")
                    for h in range(HPC):
                        nc.tensor.matmul(
                            sacc[:, h, off:512],
                            kt[h * HD:(h + 1) * HD, ki * 128:(ki + 1) * 128],
                            qt[h * HD:(h + 1) * HD, qc * 512 + off:(qc + 1) * 512],
                            start=True, stop=True)
                    nc.scalar.activation(
                        p[:, ki, :, off:512],
                        sacc[:, :, off:512], EXP, scale=SCALE)
                    if ki >= 4 * qc:  # diagonal: post-exp 0/1 mask per head
                        for h in range(HPC):
                            nc.gpsimd.tensor_mul(
                                p[:, ki, h, off:off + 128],
                                p[:, ki, h, off:off + 128],
                                tri_sb[:])
                    if fillers:
                        fillers.pop(0)()
                return p

            def attn_av(st, ctxt, drows, p, qc, fillers):
                nki = 4 * qc + 4
                caccs = [ps_cacc.tile([HD + 1, 512], F32, tag=f"c{h}",
                                      name=f"cacc{h}", bufs=1)
                         for h in range(HPC)]
                for ki in range(nki):
                    off = max(0, ki * 128 - qc * 512)
                    for h in range(HPC):
                        nc.tensor.matmul(
                            caccs[h][:, off:512],
                            st["v"][:, ki, h, :],
                            p[:, ki, h, off:512],
                            start=(ki == 0), stop=(ki == nki - 1))
                    if fillers:
                        fillers.pop(0)()
                for h in range(HPC):
                    nc.vector.tensor_copy(
                        drows[h][0:1, qc * 512:(qc + 1) * 512],
                        caccs[h][HD:HD + 1, :])
                    nc.vector.tensor_copy(
                        ctxt[h * HD:(h + 1) * HD, qc * 512:(qc + 1) * 512],
                        caccs[h][0:HD, :])

            def recip_rows(drows_h, rbf_h, c0, c1):
                # ACT-table reciprocal (~1e-5 rel err, fine for softmax
                # denominators that land in bf16 anyway). bass's
                # activation() hard-blocks Reciprocal, so emit directly.
                eng = nc.scalar
                ins = [eng.lower_ap(drows_h[0:1, c0:c1]),
                       mybir.ImmediateValue(dtype=mybir.dt.float32, value=0.0),
                       mybir.ImmediateValue(dtype=mybir.dt.float32, value=1.0),
                       mybir.ImmediateValue(dtype=mybir.dt.float32, value=0.0)]
                eng.add_instruction(mybir.InstActivation(
                    name=nc.get_next_instruction_name(),
                    func=mybir.ActivationFunctionType.Reciprocal,
                    ins=ins, outs=[eng.lower_ap(rbf_h[0:1, c0:c1])]))

            def norm_seg_psum(ctxt, rbf, qc):
                # low-latency rank-1 PE broadcast of the recip row
                # (stationary = ones column from tri row 0, K=1), then
                # per-head DVE multiplies straight off psum (DVE tolerates
                # the psum->sbuf partition shift).
                for h in range(HPC):
                    bc = ps_mm.tile([128, 512], F32, tag="mm", name="bc")
                    nc.tensor.matmul(
                        bc[0:HD, :], tri_sb[0:1, 0:HD],
                        rbf[h][0:1, qc * 512:(qc + 1) * 512],
                        start=True, stop=True)
                    sl = ctxt[h * HD:(h + 1) * HD, qc * 512:(qc + 1) * 512]
                    nc.vector.tensor_mul(sl, sl, bc[0:HD, :])

            def outproj_unit(ctxt, b, j, seg):
                oacc = ps_mm.tile([128, 512], F32, tag="mm")
                nc.tensor.matmul(oacc[:],
                                 wo_sb[:, j, :],
                                 ctxt[:, seg * 512:(seg + 1) * 512],
                                 start=True, stop=True)
                osb = outsb.tile([128, 512], BF16, tag="o")
                nc.vector.tensor_copy(osb[:], oacc[:])
                nc.sync.dma_start(
                    out_d.ap()[b, j * 128:(j + 1) * 128,
                               seg * 512:(seg + 1) * 512],
                    osb[:])

            def build_fillers(ctxt, b, rbf):
                """Deferred epilogue of batch b, popped into batch b+1's PE
                stream: 4 normalize units (rank-1 bc + DVE muls) followed by
                32 out-projection units."""
                fs = [lambda qc=qc: norm_seg_psum(ctxt, rbf, qc)
                      for qc in range(NQC)]
                fs += [lambda j=j, seg=seg: outproj_unit(ctxt, b, j, seg)
                       for seg in range(NQC) for j in range(NCH)]
                return fs

            # ---- prologue: batch 0 QKV standalone ----
            xt_sb = load_xt(0, fine=True)
            xt_next = load_xt(1)
            seqs = {0: new_seq(0)}
            for sc in range(NQC):
                qkv_chunk(seqs[0], xt_sb, sc)
            xt_sb = xt_next

            prev = None   # (ctxt of b-1, b-1) with outproj not yet emitted
            for b in range(B):
                st = seqs.pop(b)
                ctxt = seqpool.tile([128, S], BF16, tag="ctxt")
                drows = [small.tile([1, S], F32, tag="drows0", name="drows0",
                                    bufs=1),
                         small.tile([1, S], F32, tag="drows1", name="drows1",
                                    bufs=1)]
                rbf = [small.tile([1, S], BF16, tag="rbf0", name="rbf0",
                                  bufs=1),
                       small.tile([1, S], BF16, tag="rbf1", name="rbf1",
                                  bufs=1)]
                fillers = build_fillers(*prev) if prev else []

                pending = None   # (p, qc) with scores emitted, AV not yet
                for qc in range(NQC):
                    if b + 1 < B:
                        if qc == 0:
                            seqs[b + 1] = new_seq(b + 1)
                        qkv_chunk(seqs[b + 1], xt_sb, qc)
                    p = attn_scores(st, qc, fillers)
                    if pending is not None:
                        attn_av(st, ctxt, drows, pending[0], pending[1], fillers)
                    pending = (p, qc)
                    if qc == 2 and b + 2 < B:
                        xt_next = load_xt(b + 2)
                attn_av(st, ctxt, drows, pending[0], pending[1], fillers)
                for f in fillers:   # leftover units of b-1
                    f()
                for h in range(HPC):
                    recip_rows(drows[h], rbf[h], 0, S)
                if b == B - 1:
                    # inline epilogue for the last batch
                    for f in build_fillers(ctxt, b, rbf):
                        f()
                else:
                    prev = (ctxt, b, rbf)
                    xt_sb = xt_next
    nc.compile()
    return nc


def _prep_inputs(x, Wq, Wk, Wv, Wo):
    bf16 = ml_dtypes.bfloat16
    xt = np.ascontiguousarray(
        np.asarray(x, dtype=np.float32).reshape(B * S, D).T).astype(bf16)
    k = np.arange(128)[:, None]
    q = np.arange(128)[None, :]
    tri = (q >= k).astype(np.float32).astype(bf16)   # allowed = q >= k
    Wq = np.asarray(Wq, dtype=np.float32)
    Wk = np.asarray(Wk, dtype=np.float32)
    Wv = np.asarray(Wv, dtype=np.float32)
    Wo = np.asarray(Wo, dtype=np.float32)
    in_maps = []
    for c in range(N_CORES):
        sl = slice(c * CSL, (c + 1) * CSL)
        in_maps.append({
            "xt": xt,
            "wq": np.ascontiguousarray(Wq[:, sl]).astype(bf16),
            "wk": np.ascontiguousarray(Wk[:, sl]).astype(bf16),
            "wv": np.ascontiguousarray(Wv[:, sl]).astype(bf16),
            "wo": np.ascontiguousarray(Wo[sl, :]).astype(bf16),
            "tri": tri,
            "ident": np.eye(128, dtype=np.float32).astype(bf16),
        })
    return in_maps


def kernel(x, Wq, Wk, Wv, Wo, bo):
    global LAST_RESULTS
    if "nc" not in _CACHE:
        _CACHE["nc"] = _build()
    nc = _CACHE["nc"]
    in_maps = _prep_inputs(x, Wq, Wk, Wv, Wo)
    res = bass_utils.run_bass_kernel_spmd(
        nc, in_maps, core_ids=list(range(N_CORES)))
    LAST_RESULTS = res
    acc = np.zeros((B, D, S), dtype=np.float32)
    for r in res.results:
        acc += r["out"].astype(np.float32)
    out = np.ascontiguousarray(acc.transpose(0, 2, 1))
    out += np.asarray(bo, dtype=np.float32)
    return out


if __name__ == "__main__":
    rng = np.random.default_rng(0)
    scale = 1.0 / np.sqrt(D)
    ins = {
        "x": rng.standard_normal((B, S, D), dtype=np.float32),
        "Wq": rng.standard_normal((D, D), dtype=np.float32) * scale,
        "Wk": rng.standard_normal((D, D), dtype=np.float32) * scale,
        "Wv": rng.standard_normal((D, D), dtype=np.float32) * scale,
        "Wo": rng.standard_normal((D, D), dtype=np.float32) * scale,
        "bo": np.zeros(D, dtype=np.float32),
    }
    out = kernel(**ins)
    print("kernel output:", out.shape, out.dtype, float(np.abs(out).mean()))


# revision 42
# speedup vs baseline: 1.0602x; 1.0261x over previous
"""Causal multi-head attention on 8 Trainium2 NeuronCores (Bass/Tile).

Problem: B=4, S=2048, D=1024, H=16 heads (HD=64), fp32, causal softmax.

Sharding (tensor parallel over heads): core c owns heads {2c, 2c+1}:
  - Wq/Wk/Wv column slices [D, 128], Wo row slice [128, D]
  - each core computes Q/K/V for its heads over the full batch, runs
    attention for its 8 (batch, head) pairs, and produces a partial
    output projection in bf16, laid out transposed as [B, D, S]; the
    host sums the 8 partials in fp32, transposes to [B, S, D], + bo.

Device dataflow (all matmuls bf16 in / fp32 PSUM accumulate):
  - x is pre-transposed on host to xt[D, B*S] bf16 so Q/K/V projections
    are weight-stationary: Q^T[c, s] = sum_d Wq[d, c] xt[d, s]. V is
    produced transposed and relaid to [s, c] via PE transpose + DVE
    copy.
  - scores are computed transposed, S^T[k, q], with the two heads packed
    onto disjoint PE row-groups (head0 K=64 at partitions 0-63, head1 at
    64-127; confirmed ~80% concurrent on HW); one ACT exp call covers
    both heads' [128, 2, 512] chunk.
  - exp'd scores stream as the AV matmul's moving operand with [V | 1]
    stationary; the ones-column gives softmax denominators in psum row
    64. softmax skips max-subtraction (scaled causal scores are ~N(0,1)).
  - causal masking: sub-diagonal blocks are never computed; diagonal
    128x128 blocks get a post-exp multiplicative 0/1 mask (Pool).
  - denominators: AV evictions drop psum row 64 into a [1, S] row per
    head; one ACT-table reciprocal per (head, batch) on the row (the
    exp<->recip ACT table swap costs ~2.6us, so strictly once per
    batch), then a rank-1 PE matmul (ones-column stationary, K=1)
    broadcasts the recip row to 64 psum partitions and a DVE multiply
    normalizes ctxt in place. The normalize units ride the filler queue
    so the recip latency hides under the next batch's QKV.
  - out^T[d, s] = sum_c Wo[c, d] ctxt[c, s]: weight-stationary out-proj
    in [128, 512] units; psum evictions alternate DVE/ACT.
  - SOFTWARE PIPELINE (engines execute streams in program order): the
    attention of batch b is emitted interleaved with the out-projection
    of batch b-1 (units zipped between score blocks) and the QKV
    projection chunks of batch b+1 (one 512-token chunk per q-chunk
    slot), so the PE never idles while ACT works through exp and the
    clock ramp (0.65/1.2/2.4 GHz pstates, 3us to max) stays hot.
    Batch 0's QKV runs standalone with the xt DMA split into 512-token
    slices so the first matmul starts ~1.5us in; batch 3 normalizes and
    projects per q-chunk right behind its AV to shrink the tail.
"""

import numpy as np
import ml_dtypes

import concourse.bass as bass
import concourse.mybir as mybir
import concourse.tile as tile
from concourse import bacc
from concourse import bass_utils

B, S, D, H, HD = 4, 2048, 1024, 16, 64
N_CORES = 8
HPC = H // N_CORES          # heads per core = 2
CSL = HPC * HD              # per-core channel slice = 128
NSB = S // 128              # 16 s-blocks per sequence
NCH = D // 128              # 8 contraction chunks
NQC = S // 512              # 4 q-chunks of 512
BF16 = mybir.dt.bfloat16
F32 = mybir.dt.float32
EXP = mybir.ActivationFunctionType.Exp
SCALE = 1.0 / float(np.sqrt(HD))

_CACHE: dict = {}
LAST_RESULTS = None  # BassKernelResults of the most recent run (for test.py)


def _build():
    nc = bacc.Bacc("TRN2", target_bir_lowering=False, debug=False,
                   num_devices=N_CORES)
    xt_d = nc.dram_tensor("xt", [D, B * S], BF16, kind="ExternalInput")
    wq_d = nc.dram_tensor("wq", [D, CSL], BF16, kind="ExternalInput")
    wk_d = nc.dram_tensor("wk", [D, CSL], BF16, kind="ExternalInput")
    wv_d = nc.dram_tensor("wv", [D, CSL], BF16, kind="ExternalInput")
    wo_d = nc.dram_tensor("wo", [CSL, D], BF16, kind="ExternalInput")
    tri_d = nc.dram_tensor("tri", [128, 128], BF16, kind="ExternalInput")
    ident_d = nc.dram_tensor("ident", [128, 128], BF16, kind="ExternalInput")
    out_d = nc.dram_tensor("out", [B, D, S], BF16, kind="ExternalOutput")

    with tile.TileContext(nc) as tc:
        with (
            tc.tile_pool(name="const", bufs=1) as cpool,
            tc.tile_pool(name="xt", bufs=2) as xtpool,
            tc.tile_pool(name="seq", bufs=2) as seqpool,
            tc.tile_pool(name="p", bufs=1) as ppool,
            tc.tile_pool(name="small", bufs=4) as small,
            tc.tile_pool(name="outsb", bufs=8) as outsb,
            tc.tile_pool(name="ps_s", bufs=2, space="PSUM") as ps_s,
            tc.tile_pool(name="ps_mm", bufs=2, space="PSUM") as ps_mm,
            tc.tile_pool(name="ps_cacc", bufs=2, space="PSUM") as ps_cacc,
        ):
            wq_sb = cpool.tile([128, NCH, CSL], BF16)
            wk_sb = cpool.tile([128, NCH, CSL], BF16)
            wv_sb = cpool.tile([128, NCH, CSL], BF16)
            wo_sb = cpool.tile([128, NCH, 128], BF16)
            tri_sb = cpool.tile([128, 128], BF16)
            ident_sb = cpool.tile([128, 128], BF16)
            # split the input transfers across BOTH HWDGE queues (sync=SP
            # and scalar=ACT rings run in parallel): dependency granularity
            # is the per-queue completion counter, so what matters is how
            # soon each queue finishes everything ahead of the consumer.
            nc.scalar.dma_start(wq_sb[:], wq_d.ap().rearrange("(c p) m -> p c m", p=128))
            nc.sync.dma_start(wk_sb[:], wk_d.ap().rearrange("(c p) m -> p c m", p=128))
            nc.scalar.dma_start(wv_sb[:], wv_d.ap().rearrange("(c p) m -> p c m", p=128))
            nc.sync.dma_start(wo_sb[:], wo_d.ap().rearrange("p (j m) -> p j m", m=128))
            nc.scalar.dma_start(tri_sb[:], tri_d.ap())
            nc.scalar.dma_start(ident_sb[:], ident_d.ap())

            def load_xt(b, split=False):
                # split=True alternates the 512KB chunks across both DMA
                # queues, halving the time until the whole tile is resident
                # (matters for batch 0, which gates the first matmuls).
                xt_sb = xtpool.tile([128, NCH, S], BF16)
                for c in range(NCH):
                    eng = nc.scalar if (split and c % 2) else nc.sync
                    eng.dma_start(
                        xt_sb[:, c, :],
                        xt_d.ap()[c * 128:(c + 1) * 128, b * S:(b + 1) * S])
                return xt_sb

            def new_seq(b):
                st = {
                    "qt": seqpool.tile([128, S], BF16, tag="qt", name="qt"),
                    "kt": seqpool.tile([128, S], BF16, tag="kt", name="kt"),
                    "vt": seqpool.tile([128, S], BF16, tag="vt", name="vt"),
                    "v": seqpool.tile([128, NSB, HPC, HD + 1], BF16, tag="v",
                                      name="v"),
                }
                nc.vector.memset(st["v"][:, :, :, HD:HD + 1], 1.0)
                return st

            def qkv_chunk(st, xt_sb, sc):
                """Q/K/V projection + V relayout for one 512-token chunk."""
                qkacc = ps_s.tile([128, 2, 512], F32, tag="s", name="qkacc")
                vacc = ps_mm.tile([128, 512], F32, tag="mm", name="vacc")
                for c in range(NCH):
                    for i, w_sb in enumerate((wq_sb, wk_sb)):
                        nc.tensor.matmul(
                            qkacc[:, i, :],
                            w_sb[:, c, :],
                            xt_sb[:, c, sc * 512:(sc + 1) * 512],
                            start=(c == 0), stop=(c == NCH - 1))
                    nc.tensor.matmul(
                        vacc[:],
                        wv_sb[:, c, :],
                        xt_sb[:, c, sc * 512:(sc + 1) * 512],
                        start=(c == 0), stop=(c == NCH - 1))
                for i, key in enumerate(("qt", "kt")):
                    nc.vector.tensor_copy(
                        st[key][:, sc * 512:(sc + 1) * 512], qkacc[:, i, :])
                nc.vector.tensor_copy(
                    st["vt"][:, sc * 512:(sc + 1) * 512], vacc[:])
                for i in range(4):
                    sb = sc * 4 + i
                    tp = ps_mm.tile([128, 128], BF16, tag="mm")
                    nc.tensor.transpose(
                        tp[:], st["vt"][:, sb * 128:(sb + 1) * 128], ident_sb[:])
                    nc.vector.tensor_copy(
                        st["v"][:, sb, :, 0:HD],
                        tp[:].rearrange("p (h e) -> p h e", h=HPC))

            # p tiles alternate between two tags (qc0/qc2 and qc1/qc3) so
            # only ~2 q-chunks of exp'd scores are resident at once.
            PTAGS = {0: ("pA", 12), 1: ("pB", 16), 2: ("pA", 12), 3: ("pB", 16)}

            def attn_scores(st, qc, fillers):
                """Score blocks for q-chunk qc; after each block, pop one
                deferred filler (out-proj unit closure) into the PE stream."""
                qt, kt = st["qt"], st["kt"]
                nki = 4 * qc + 4   # causal: k-blocks 0 .. 4qc+3
                tag, maxk = PTAGS[qc]
                p = ppool.tile([128, maxk, HPC, 512], BF16, tag=tag, name=tag)
                for ki in range(nki):
                    off = max(0, ki * 128 - qc * 512)
                    sacc = ps_s.tile([128, 2, 512], F32, tag="s")
                    for h in range(HPC):
                        nc.tensor.matmul(
                            sacc[:, h, off:512],
                            kt[h * HD:(h + 1) * HD, ki * 128:(ki + 1) * 128],
                            qt[h * HD:(h + 1) * HD, qc * 512 + off:(qc + 1) * 512],
                            start=True, stop=True)
                    nc.scalar.activation(
                        p[:, ki, :, off:512],
                        sacc[:, :, off:512], EXP, scale=SCALE)
                    if ki >= 4 * qc:  # diagonal: post-exp 0/1 mask per head
                        for h in range(HPC):
                            nc.gpsimd.tensor_mul(
                                p[:, ki, h, off:off + 128],
                                p[:, ki, h, off:off + 128],
                                tri_sb[:])
                    if fillers:
                        fillers.pop(0)()
                return p

            def attn_av(st, ctxt, drows, p, qc, fillers):
                nki = 4 * qc + 4
                caccs = [ps_cacc.tile([HD + 1, 512], F32, tag=f"c{h}",
                                      name=f"cacc{h}", bufs=1)
                         for h in range(HPC)]
                for ki in range(nki):
                    off = max(0, ki * 128 - qc * 512)
                    for h in range(HPC):
                        nc.tensor.matmul(
                            caccs[h][:, off:512],
                            st["v"][:, ki, h, :],
                            p[:, ki, h, off:512],
                            start=(ki == 0), stop=(ki == nki - 1))
                    if fillers:
                        fillers.pop(0)()
                for h in range(HPC):
                    nc.vector.tensor_copy(
                        drows[h][0:1, qc * 512:(qc + 1) * 512],
                        caccs[h][HD:HD + 1, :])
                    nc.vector.tensor_copy(
                        ctxt[h * HD:(h + 1) * HD, qc * 512:(qc + 1) * 512],
                        caccs[h][0:HD, :])

            def recip_rows(drows_h, rbf_h, c0, c1):
                # ACT-table reciprocal (~1e-5 rel err, fine for softmax
                # denominators that land in bf16 anyway). bass's
                # activation() hard-blocks Reciprocal, so emit directly.
                eng = nc.scalar
                ins = [eng.lower_ap(drows_h[0:1, c0:c1]),
                       mybir.ImmediateValue(dtype=mybir.dt.float32, value=0.0),
                       mybir.ImmediateValue(dtype=mybir.dt.float32, value=1.0),
                       mybir.ImmediateValue(dtype=mybir.dt.float32, value=0.0)]
                eng.add_instruction(mybir.InstActivation(
                    name=nc.get_next_instruction_name(),
                    func=mybir.ActivationFunctionType.Reciprocal,
                    ins=ins, outs=[eng.lower_ap(rbf_h[0:1, c0:c1])]))

            def norm_seg_psum(ctxt, rbf, qc):
                # low-latency rank-1 PE broadcast of the recip row
                # (stationary = ones column from tri row 0, K=1), then
                # per-head DVE multiplies straight off psum (DVE tolerates
                # the psum->sbuf partition shift).
                for h in range(HPC):
                    bc = ps_mm.tile([128, 512], F32, tag="mm", name="bc")
                    nc.tensor.matmul(
                        bc[0:HD, :], tri_sb[0:1, 0:HD],
                        rbf[h][0:1, qc * 512:(qc + 1) * 512],
                        start=True, stop=True)
                    sl = ctxt[h * HD:(h + 1) * HD, qc * 512:(qc + 1) * 512]
                    nc.vector.tensor_mul(sl, sl, bc[0:HD, :])

            def outproj_unit(ctxt, b, j, seg, alt=False):
                oacc = ps_mm.tile([128, 512], F32, tag="mm")
                nc.tensor.matmul(oacc[:],
                                 wo_sb[:, j, :],
                                 ctxt[:, seg * 512:(seg + 1) * 512],
                                 start=True, stop=True)
                osb = outsb.tile([128, 512], BF16, tag="o")
                if alt and j % 2:
                    # last batch only: exp is finished, so ACT can help
                    # drain the epilogue evictions
                    nc.scalar.copy(osb[:], oacc[:])
                else:
                    nc.vector.tensor_copy(osb[:], oacc[:])
                (nc.scalar if (alt and j % 2 == 0) else nc.sync).dma_start(
                    out_d.ap()[b, j * 128:(j + 1) * 128,
                               seg * 512:(seg + 1) * 512],
                    osb[:])

            def build_fillers(ctxt, b, rbf, alt=False):
                """Deferred epilogue of batch b, popped into batch b+1's PE
                stream: 4 normalize units (rank-1 bc + DVE muls) followed by
                32 out-projection units."""
                fs = [lambda qc=qc: norm_seg_psum(ctxt, rbf, qc)
                      for qc in range(NQC)]
                fs += [lambda j=j, seg=seg: outproj_unit(ctxt, b, j, seg, alt)
                       for seg in range(NQC) for j in range(NCH)]
                return fs

            # ---- prologue: batch 0 QKV standalone ----
            xt_sb = load_xt(0, split=True)
            xt_next = load_xt(1)
            seqs = {0: new_seq(0)}
            for sc in range(NQC):
                qkv_chunk(seqs[0], xt_sb, sc)
            xt_sb = xt_next

            prev = None   # (ctxt of b-1, b-1) with outproj not yet emitted
            for b in range(B):
                st = seqs.pop(b)
                ctxt = seqpool.tile([128, S], BF16, tag="ctxt")
                drows = [small.tile([1, S], F32, tag="drows0", name="drows0",
                                    bufs=1),
                         small.tile([1, S], F32, tag="drows1", name="drows1",
                                    bufs=1)]
                rbf = [small.tile([1, S], BF16, tag="rbf0", name="rbf0",
                                  bufs=1),
                       small.tile([1, S], BF16, tag="rbf1", name="rbf1",
                                  bufs=1)]
                fillers = build_fillers(*prev) if prev else []

                pending = None   # (p, qc) with scores emitted, AV not yet
                for qc in range(NQC):
                    if b + 1 < B:
                        if qc == 0:
                            seqs[b + 1] = new_seq(b + 1)
                        qkv_chunk(seqs[b + 1], xt_sb, qc)
                    p = attn_scores(st, qc, fillers)
                    if pending is not None:
                        attn_av(st, ctxt, drows, pending[0], pending[1], fillers)
                    pending = (p, qc)
                    if qc == 2 and b + 2 < B:
                        xt_next = load_xt(b + 2)
                attn_av(st, ctxt, drows, pending[0], pending[1], fillers)
                for f in fillers:   # leftover units of b-1
                    f()
                for h in range(HPC):
                    recip_rows(drows[h], rbf[h], 0, S)
                if b == B - 1:
                    # inline epilogue for the last batch
                    for f in build_fillers(ctxt, b, rbf, alt=True):
                        f()
                else:
                    prev = (ctxt, b, rbf)
                    xt_sb = xt_next
    nc.compile()
    return nc


def _prep_inputs(x, Wq, Wk, Wv, Wo):
    bf16 = ml_dtypes.bfloat16
    xt = np.ascontiguousarray(
        np.asarray(x, dtype=np.float32).reshape(B * S, D).T).astype(bf16)
    k = np.arange(128)[:, None]
    q = np.arange(128)[None, :]
    tri = (q >= k).astype(np.float32).astype(bf16)   # allowed = q >= k
    Wq = np.asarray(Wq, dtype=np.float32)
    Wk = np.asarray(Wk, dtype=np.float32)
    Wv = np.asarray(Wv, dtype=np.float32)
    Wo = np.asarray(Wo, dtype=np.float32)
    in_maps = []
    for c in range(N_CORES):
        sl = slice(c * CSL, (c + 1) * CSL)
        in_maps.append({
            "xt": xt,
            "wq": np.ascontiguousarray(Wq[:, sl]).astype(bf16),
            "wk": np.ascontiguousarray(Wk[:, sl]).astype(bf16),
            "wv": np.ascontiguousarray(Wv[:, sl]).astype(bf16),
            "wo": np.ascontiguousarray(Wo[sl, :]).astype(bf16),
            "tri": tri,
            "ident": np.eye(128, dtype=np.float32).astype(bf16),
        })
    return in_maps


def kernel(x, Wq, Wk, Wv, Wo, bo):
    global LAST_RESULTS
    if "nc" not in _CACHE:
        _CACHE["nc"] = _build()
    nc = _CACHE["nc"]
    in_maps = _prep_inputs(x, Wq, Wk, Wv, Wo)
    res = bass_utils.run_bass_kernel_spmd(
        nc, in_maps, core_ids=list(range(N_CORES)))
    LAST_RESULTS = res
    acc = np.zeros((B, D, S), dtype=np.float32)
    for r in res.results:
        acc += r["out"].astype(np.float32)
    out = np.ascontiguousarray(acc.transpose(0, 2, 1))
    out += np.asarray(bo, dtype=np.float32)
    return out


if __name__ == "__main__":
    rng = np.random.default_rng(0)
    scale = 1.0 / np.sqrt(D)
    ins = {
        "x": rng.standard_normal((B, S, D), dtype=np.float32),
        "Wq": rng.standard_normal((D, D), dtype=np.float32) * scale,
        "Wk": rng.standard_normal((D, D), dtype=np.float32) * scale,
        "Wv": rng.standard_normal((D, D), dtype=np.float32) * scale,
        "Wo": rng.standard_normal((D, D), dtype=np.float32) * scale,
        "bo": np.zeros(D, dtype=np.float32),
    }
    out = kernel(**ins)
    print("kernel output:", out.shape, out.dtype, float(np.abs(out).mean()))
